# revision 19
# baseline (speedup 1.0000x reference)
"""Dynamic-conv (CondConv-style) kernel for Trainium2, 8 NeuronCores.

Problem: for each sample b:
    se     = global-avg-pool(x[b])                     (256,)
    gates  = sigmoid(se @ route_w.T + route_b)         (8,)
    w_dyn  = (gates @ weight.T).reshape(256,256,3,3)   per-sample 3x3 conv kernel
    out[b] = conv2d(x[b], w_dyn, pad=1) + bias         (256,28,28)

Sharding: data-parallel over batch, 4 samples per core; the expert weight
bank and routing weights are replicated.

Per-core plan:
  - x: DMA fp32 -> cast+zero-pad to bf16 (30x30) on DVE, fused free-dim
    sum (accum_out) to get the pooled `se`.
  - gates: tiny fp32 matmuls against a 16x-replicated routing matrix produce
    a 128x64 block of logits; sigmoid (ACT) + block-diagonal mask (DVE)
    yield a block-diagonal stationary G[(q,e),(s,q)] = gate[s,e] * (q==q').
  - kernel synthesis on the TensorEngine: the weight bank is laid out so a
    rhs tile's partition p=(q,e) carries 16 different ci-chunks of all 8
    experts; one matmul against G computes sum_e gate[s,e]*W_e for 16
    chunks x 4 samples at once (128-wide contraction, 128 elems/cycle).
  - PSUM (64,384) tiles drain (cast bf16) alternately on ACT/DVE into a
    (64, 16*3*384) SBUF stage; one SBUF->SBUF DMA per (sample, ci_tile)
    re-gathers w_dyn with ci on partitions -- the conv stationary layout.
    M is ordered sample-major so the re-gather source is a contiguous
    8-partition slice.
  - conv: 9 shifted matmuls (bf16) accumulating over (ci_tile, kh, kw) in
    PSUM; ACT adds bias on the PSUM->SBUF drain; DMA out fp32.
  - split into halves by output-channel block so conv(half A) overlaps the
    synthesis of half B; DMA issue is spread over sync/scalar/vector queues.
"""

import os
from contextlib import ExitStack

import ml_dtypes
import numpy as np

import concourse.bacc as bacc
import concourse.bass as bass
import concourse.mybir as mybir
import concourse.tile as tile
from concourse.bass_utils import run_bass_kernel_spmd

FP32 = mybir.dt.float32
BF16 = mybir.dt.bfloat16
BF16_NP = ml_dtypes.bfloat16

N_CORES = 8
B, C_IN, H, W = 32, 256, 28, 28
NUM, C_OUT, K = 8, 256, 3
BS = B // N_CORES          # samples per core = 4
NQ = 16                    # ci chunks in the synthesis contraction
F = 2304                   # f = co_t*1152 + khkw*128 + co_lo
NWIN = 384                 # synthesis matmul free size


def build_nc() -> bacc.Bacc:
    nc = bacc.Bacc("TRN2", target_bir_lowering=False, debug=False,
                   num_devices=N_CORES)

    x_d = nc.dram_tensor("x", [BS, C_IN, H, W], BF16, kind="ExternalInput")
    # bank[p=(q,e), w, cl, n] = W[e, ci=q*16+cl, f=w*384+n]; per-partition
    # contiguous so one DMA per w moves 12.3KB/partition runs.
    bank_d = nc.dram_tensor("bank", [128, 6, NQ, NWIN], BF16, kind="ExternalInput")
    rwx_d = nc.dram_tensor("rwx", [C_IN, 128], FP32, kind="ExternalInput")
    rbx_d = nc.dram_tensor("rbx", [128, 1], FP32, kind="ExternalInput")
    mask_d = nc.dram_tensor("mask", [128, 64], BF16, kind="ExternalInput")
    bias_d = nc.dram_tensor("bias", [2, 128, 1], FP32, kind="ExternalInput")
    out_d = nc.dram_tensor("out", [BS, C_OUT, H, W], FP32, kind="ExternalOutput")
    dbg = os.environ.get("KERNEL_DEBUG_TAPS")
    if dbg:
        dbgG_d = nc.dram_tensor("dbgG", [128, 64], BF16, kind="ExternalOutput")
        dbgS_d = nc.dram_tensor("dbgS", [128, 8, 3, 384], BF16, kind="ExternalOutput")
        dbgW_d = nc.dram_tensor("dbgW", [128, 9, 128], BF16, kind="ExternalOutput")

    with tile.TileContext(nc) as tc, ExitStack() as ctx:
        singles = ctx.enter_context(tc.tile_pool(name="singles", bufs=1))
        xstage = ctx.enter_context(tc.tile_pool(name="xstage", bufs=3))
        xpadp = ctx.enter_context(tc.tile_pool(name="xpadp", bufs=1))
        bankp = ctx.enter_context(tc.tile_pool(name="bankp", bufs=2))
        stagep = ctx.enter_context(tc.tile_pool(name="stagep", bufs=2))
        wdynp = ctx.enter_context(tc.tile_pool(name="wdynp", bufs=1))
        outp = ctx.enter_context(tc.tile_pool(name="outp", bufs=2))
        psS = ctx.enter_context(tc.tile_pool(name="psS", bufs=4, space="PSUM"))
        psC = ctx.enter_context(tc.tile_pool(name="psC", bufs=4, space="PSUM"))

        # ---- replicated constants
        rwx = singles.tile([128, 2, 128], FP32)
        for t in range(2):
            nc.sync.dma_start(out=rwx[:, t, :], in_=rwx_d[t * 128:(t + 1) * 128, :])
        rbx = singles.tile([128, 1], FP32)
        nc.sync.dma_start(out=rbx, in_=rbx_d[:])
        mask = singles.tile([128, 64], BF16)
        nc.sync.dma_start(out=mask, in_=mask_d[:])
        biasT = singles.tile([128, 2], FP32)
        for t in range(2):
            nc.sync.dma_start(out=biasT[:, t:t + 1], in_=bias_d[t])
        ones16 = singles.tile([128, NQ], FP32)
        nc.vector.memset(ones16, 1.0)
        warm = singles.tile([128, 1], FP32)
        nc.scalar.activation(out=warm, in_=ones16[:, 0:1],
                             func=mybir.ActivationFunctionType.Sigmoid)
        nc.scalar.activation(out=warm, in_=warm,
                             func=mybir.ActivationFunctionType.Identity,
                             bias=warm, scale=1.0)
        nc.scalar.activation(out=warm, in_=warm,
                             func=mybir.ActivationFunctionType.Copy)

        # ---- x in; cast+pad to bf16 with fused pooling
        se = singles.tile([128, 2, BS], FP32)
        xp = {}
        for s in range(BS):
            for t in range(2):
                xs = xstage.tile([128, H, W], BF16, tag="xs")
                nc.scalar.dma_start(out=xs, in_=x_d[s, t * 128:(t + 1) * 128])
                xpt = xpadp.tile([128, H + 2, W + 2], BF16, tag=f"xp{s}{t}")
                nc.vector.memset(xpt, 0.0)
                nc.vector.tensor_scalar(
                    out=xpt[:, 1:H + 1, 1:W + 1],
                    in0=xs,
                    scalar1=1.0,
                    scalar2=None,
                    op0=mybir.AluOpType.mult,
                    op1=mybir.AluOpType.add,
                    accum_out=se[:, t, s:s + 1],
                )
                xp[s, t] = xpt

        # ---- gates -> block-diagonal stationary G (M is (s, q) sample-major)
        se_rep = singles.tile([128, 2, 64], FP32)
        for t in range(2):
            for s in range(BS):
                nc.vector.tensor_scalar(
                    out=se_rep[:, t, NQ * s:NQ * (s + 1)], in0=ones16,
                    scalar1=se[:, t, s:s + 1], scalar2=None,
                    op0=mybir.AluOpType.mult)
        L = psS.tile([128, 64], FP32, tag="ps", name="Lpsum")
        for t in range(2):
            nc.tensor.matmul(L, lhsT=rwx[:, t, :], rhs=se_rep[:, t, :],
                             start=(t == 0), stop=(t == 1))
        g0 = singles.tile([128, 64], FP32)
        nc.scalar.activation(out=g0, in_=L,
                             func=mybir.ActivationFunctionType.Sigmoid,
                             bias=rbx, scale=1.0)
        G = singles.tile([128, 64], BF16)
        nc.vector.tensor_tensor(out=G, in0=g0, in1=mask, op=mybir.AluOpType.mult)
        if dbg:
            nc.sync.dma_start(out=dbgG_d[:], in_=G)

        # ---- per half: synthesize w_dyn on PE, SBUF re-gather, then conv
        for half in range(2):
            stage = stagep.tile([128, 8, 3, NWIN], BF16, tag="stage",
                                name=f"stage{half}")
            for wloc in range(3):
                w = 3 * half + wloc
                bks = []
                for u in range(2):
                    bk = bankp.tile([128, 8, NWIN], BF16, tag="bk",
                                    name=f"bk{half}_{wloc}_{u}")
                    nc.sync.dma_start(out=bk, in_=bank_d[:, w, 8 * u:8 * u + 8])
                    bks.append(bk)
                for clp in range(8):
                    # MMs for cl=clp (u=0) and cl=clp+8 (u=1) share one
                    # (128,384) PSUM tile via column-strip offsets; one
                    # drain covers both
                    ps = psS.tile([128, NWIN], FP32, tag="ps",
                                  name=f"ps{half}_{wloc}_{clp}")
                    nc.tensor.matmul(ps[0:64, :], lhsT=G,
                                     rhs=bks[0][:, clp, :],
                                     start=True, stop=True)
                    nc.tensor.matmul(ps[64:128, :], lhsT=G,
                                     rhs=bks[1][:, clp, :],
                                     start=True, stop=True)
                    dst = stage[:, clp, wloc, :]
                    if clp % 2 == 0:
                        nc.scalar.activation(
                            out=dst, in_=ps,
                            func=mybir.ActivationFunctionType.Copy)
                    else:
                        nc.vector.tensor_copy(out=dst, in_=ps)

            if dbg and half == 0:
                nc.sync.dma_start(out=dbgS_d[:], in_=stage)
            wd = {}
            for s in range(BS):
                for t in range(2):
                    wdt = wdynp.tile([128, 9, 128], BF16, tag=f"wd{half}{s}{t}")
                    # stage partition 64u+16s+8t+q' holds ci=(8t+q')*16+clp+8u
                    # for clp in the free dim; with the host-side ci
                    # permutation, partition d=64u+8q'+clp of the conv
                    # stationary IS that channel, so both dst slices are
                    # contiguous.
                    for u in range(2):
                        src = stage[64 * u + NQ * s + 8 * t:
                                    64 * u + NQ * s + 8 * t + 8]
                        eng = nc.sync if u == 0 else nc.scalar
                        eng.dma_start(out=wdt[64 * u:64 * (u + 1)], in_=src)
                    wd[s, t] = wdt
                    if dbg and half == 0 and s == 0 and t == 0:
                        nc.sync.dma_start(out=dbgW_d[:], in_=wdt)

            for s in range(BS):
                pst = [psC.tile([128, 14, W], FP32, tag="pc",
                                name=f"pc{half}_{s}_{c}") for c in range(2)]
                for t in range(2):
                    for k in range(9):
                        kh, kw = divmod(k, 3)
                        lw = wd[s, t][:, k, :]
                        for c in range(2):
                            rhs = xp[s, t][:, c * 14 + kh:c * 14 + kh + 14, kw:kw + W]
                            nc.tensor.matmul(
                                pst[c], lhsT=lw, rhs=rhs,
                                start=(t == 0 and k == 0),
                                stop=(t == 1 and k == 8),
                            )
                ot = outp.tile([128, 2, 14, W], FP32, tag="ot",
                               name=f"ot{half}_{s}")
                for c in range(2):
                    nc.scalar.activation(
                        out=ot[:, c], in_=pst[c],
                        func=mybir.ActivationFunctionType.Identity,
                        bias=biasT[:, half:half + 1], scale=1.0)
                nc.scalar.dma_start(
                    out=out_d[s, half * 128:(half + 1) * 128], in_=ot)
    nc.finalize()
    return nc


# partition d (within a 128-channel tile) holds channel perm[d]:
# d = 64u + 8q' + clp  <->  ci_lo = 16q' + 8u + clp
CI_PERM = np.array([(d % 64) // 8 * 16 + (d // 64) * 8 + d % 8
                    for d in range(128)])
CI_MAP = np.concatenate([CI_PERM, 128 + CI_PERM])


def _host_prep(route_w, route_b, weight, bias):
    """Host-side layout transforms (pure numpy, replicated to every core)."""
    We = np.ascontiguousarray(weight.T).reshape(NUM, C_OUT, C_IN, K, K)
    Wf = We.transpose(0, 2, 1, 3, 4)            # [e, ci, co, kh, kw]
    Wf = Wf.reshape(NUM, C_IN, 2, 128, 9)       # [e, ci, co_t, co_lo, khkw]
    Wf = Wf.transpose(0, 1, 2, 4, 3)            # [e, ci, co_t, khkw, co_lo]
    Wf = Wf.reshape(NUM, C_IN, F)               # f = co_t*1152 + khkw*128 + co_lo
    Bk = Wf.reshape(NUM, NQ, NQ, 6, NWIN)       # [e, q, cl, w, n]
    bank = np.ascontiguousarray(
        Bk.transpose(1, 0, 3, 2, 4).reshape(128, 6, NQ, NWIN)).astype(BF16_NP)

    rwx = np.ascontiguousarray(
        np.tile((route_w / (H * W)).T, (1, NQ))[CI_MAP]).astype(np.float32)
    rbx = np.tile(route_b, NQ).reshape(128, 1).astype(np.float32)
    # G column m = (s, q): q(m) = m % 16
    mask = (np.arange(128)[:, None] // 8 == np.arange(64)[None, :] % NQ
            ).astype(BF16_NP)
    bias2 = np.ascontiguousarray(bias.reshape(2, 128, 1)).astype(np.float32)
    return bank, rwx, rbx, mask, bias2


def _ensure_ntff_hook():
    """Provide antenv.axon_hooks (absent in this image) so trace=True works.

    The boot script ships a ctypes NTFF hook but can only register it through
    antenv.axon_hooks; shim that module and register the hook ourselves.
    """
    import sys
    import types
    try:
        from antenv.axon_hooks import get_axon_ntff_profile_hook  # noqa: F401
        return
    except ImportError:
        pass
    try:
        import antenv
        from trn_agent_boot.trn_boot import _ntff_profile_via_ctypes
    except ImportError:
        return
    mod = types.ModuleType("antenv.axon_hooks")
    holder = {"hook": None}
    mod.set_axon_ntff_profile_hook = lambda h: holder.__setitem__("hook", h)
    mod.get_axon_ntff_profile_hook = lambda: holder["hook"]
    sys.modules["antenv.axon_hooks"] = mod
    antenv.axon_hooks = mod
    mod.set_axon_ntff_profile_hook(
        _ntff_profile_via_ctypes("/opt/axon/libaxon_pjrt.so"))


_NC_CACHE = None


def kernel(inputs, route_w, route_b, weight, bias):
    global _NC_CACHE
    inputs = np.asarray(inputs, dtype=np.float32)
    route_w = np.asarray(route_w, dtype=np.float32)
    route_b = np.asarray(route_b, dtype=np.float32)
    weight = np.asarray(weight, dtype=np.float32)
    bias = np.asarray(bias, dtype=np.float32)

    bank, rwx, rbx, mask, bias2 = _host_prep(route_w, route_b, weight, bias)

    if _NC_CACHE is None:
        _NC_CACHE = build_nc()
    nc = _NC_CACHE

    shared = {"bank": bank, "rwx": rwx, "rbx": rbx, "mask": mask, "bias": bias2}
    x16 = inputs[:, CI_MAP].astype(BF16_NP)
    in_maps = [
        {"x": np.ascontiguousarray(x16[BS * c:BS * (c + 1)]), **shared}
        for c in range(N_CORES)
    ]
    trace = bool(int(os.environ.get("KERNEL_TRACE", "0")))
    if trace:
        _ensure_ntff_hook()
    res = run_bass_kernel_spmd(
        nc, in_maps, core_ids=list(range(N_CORES)), trace=trace,
        tmpdir=os.environ.get("KERNEL_TMPDIR"),
    )
    out = np.concatenate([res.results[c]["out"] for c in range(N_CORES)], axis=0)
    kernel.last_results = res
    return out


kernel.last_results = None


# revision 20
# speedup vs baseline: 1.1568x; 1.1568x over previous
"""Dynamic-conv (CondConv-style) kernel for Trainium2, 8 NeuronCores.

Problem: for each sample b:
    se     = global-avg-pool(x[b])                     (256,)
    gates  = sigmoid(se @ route_w.T + route_b)         (8,)
    w_dyn  = (gates @ weight.T).reshape(256,256,3,3)   per-sample 3x3 conv kernel
    out[b] = conv2d(x[b], w_dyn, pad=1) + bias         (256,28,28)

Sharding: data-parallel over batch, 4 samples per core; the expert weight
bank and routing weights are replicated.

Per-core plan:
  - x: DMA fp32 -> cast+zero-pad to bf16 (30x30) on DVE, fused free-dim
    sum (accum_out) to get the pooled `se`.
  - gates: tiny fp32 matmuls against a 16x-replicated routing matrix produce
    a 128x64 block of logits; sigmoid (ACT) + block-diagonal mask (DVE)
    yield a block-diagonal stationary G[(q,e),(s,q)] = gate[s,e] * (q==q').
  - kernel synthesis on the TensorEngine: the weight bank is laid out so a
    rhs tile's partition p=(q,e) carries 16 different ci-chunks of all 8
    experts; one matmul against G computes sum_e gate[s,e]*W_e for 16
    chunks x 4 samples at once (128-wide contraction, 128 elems/cycle).
  - PSUM (64,384) tiles drain (cast bf16) alternately on ACT/DVE into a
    (64, 16*3*384) SBUF stage; one SBUF->SBUF DMA per (sample, ci_tile)
    re-gathers w_dyn with ci on partitions -- the conv stationary layout.
    M is ordered sample-major so the re-gather source is a contiguous
    8-partition slice.
  - conv: 9 shifted matmuls (bf16) accumulating over (ci_tile, kh, kw) in
    PSUM; ACT adds bias on the PSUM->SBUF drain; DMA out fp32.
  - split into halves by output-channel block so conv(half A) overlaps the
    synthesis of half B; DMA issue is spread over sync/scalar/vector queues.
"""

import os
from contextlib import ExitStack

import ml_dtypes
import numpy as np

import concourse.bacc as bacc
import concourse.bass as bass
import concourse.mybir as mybir
import concourse.tile as tile
from concourse.bass_utils import run_bass_kernel_spmd

FP32 = mybir.dt.float32
BF16 = mybir.dt.bfloat16
BF16_NP = ml_dtypes.bfloat16

N_CORES = 8
B, C_IN, H, W = 32, 256, 28, 28
NUM, C_OUT, K = 8, 256, 3
BS = B // N_CORES          # samples per core = 4
NQ = 16                    # ci chunks in the synthesis contraction
F = 2304                   # f = co_t*1152 + khkw*128 + co_lo
NWIN = 384                 # synthesis matmul free size


def build_nc() -> bacc.Bacc:
    nc = bacc.Bacc("TRN2", target_bir_lowering=False, debug=False,
                   num_devices=N_CORES)

    x_d = nc.dram_tensor("x", [BS, C_IN, H, W], BF16, kind="ExternalInput")
    # bank[p=(q,e), w, cl, n] = W[e, ci=q*16+cl, f=w*384+n]; per-partition
    # contiguous so one DMA per w moves 12.3KB/partition runs.
    bank_d = nc.dram_tensor("bank", [128, 6, NQ, NWIN], BF16, kind="ExternalInput")
    rwx_d = nc.dram_tensor("rwx", [C_IN, 128], FP32, kind="ExternalInput")
    rbx_d = nc.dram_tensor("rbx", [128, 1], FP32, kind="ExternalInput")
    mask_d = nc.dram_tensor("mask", [128, 64], BF16, kind="ExternalInput")
    bias_d = nc.dram_tensor("bias", [2, 128, 1], FP32, kind="ExternalInput")
    out_d = nc.dram_tensor("out", [BS, C_OUT, H, W], FP32, kind="ExternalOutput")
    dbg = os.environ.get("KERNEL_DEBUG_TAPS")
    if dbg:
        dbgG_d = nc.dram_tensor("dbgG", [128, 64], BF16, kind="ExternalOutput")
        dbgS_d = nc.dram_tensor("dbgS", [128, 8, 3, 384], BF16, kind="ExternalOutput")
        dbgW_d = nc.dram_tensor("dbgW", [128, 9, 128], BF16, kind="ExternalOutput")

    with tile.TileContext(nc) as tc, ExitStack() as ctx:
        singles = ctx.enter_context(tc.tile_pool(name="singles", bufs=1))
        xstage = ctx.enter_context(tc.tile_pool(name="xstage", bufs=3))
        xpadp = ctx.enter_context(tc.tile_pool(name="xpadp", bufs=1))
        bankp = ctx.enter_context(tc.tile_pool(name="bankp", bufs=4))
        stagep = ctx.enter_context(tc.tile_pool(name="stagep", bufs=2))
        wdynp = ctx.enter_context(tc.tile_pool(name="wdynp", bufs=1))
        outp = ctx.enter_context(tc.tile_pool(name="outp", bufs=2))
        psS = ctx.enter_context(tc.tile_pool(name="psS", bufs=4, space="PSUM"))
        psC = ctx.enter_context(tc.tile_pool(name="psC", bufs=4, space="PSUM"))

        # ---- replicated constants
        rwx = singles.tile([128, 2, 128], FP32)
        for t in range(2):
            nc.sync.dma_start(out=rwx[:, t, :], in_=rwx_d[t * 128:(t + 1) * 128, :])
        rbx = singles.tile([128, 1], FP32)
        nc.sync.dma_start(out=rbx, in_=rbx_d[:])
        mask = singles.tile([128, 64], BF16)
        nc.sync.dma_start(out=mask, in_=mask_d[:])
        biasT = singles.tile([128, 2], FP32)
        for t in range(2):
            nc.sync.dma_start(out=biasT[:, t:t + 1], in_=bias_d[t])
        ones16 = singles.tile([128, NQ], FP32)
        nc.vector.memset(ones16, 1.0)
        warm = singles.tile([128, 1], FP32)
        nc.scalar.activation(out=warm, in_=ones16[:, 0:1],
                             func=mybir.ActivationFunctionType.Sigmoid)
        nc.scalar.activation(out=warm, in_=warm,
                             func=mybir.ActivationFunctionType.Identity,
                             bias=warm, scale=1.0)
        nc.scalar.activation(out=warm, in_=warm,
                             func=mybir.ActivationFunctionType.Copy)

        # ---- x in; cast+pad to bf16 with fused pooling
        se = singles.tile([128, 2, BS], FP32)
        xp = {}
        for s in range(BS):
            for t in range(2):
                xs = xstage.tile([128, H, W], BF16, tag="xs")
                nc.scalar.dma_start(out=xs, in_=x_d[s, t * 128:(t + 1) * 128])
                xpt = xpadp.tile([128, H + 2, W + 2], BF16, tag=f"xp{s}{t}")
                nc.gpsimd.memset(xpt, 0.0)
                nc.vector.tensor_scalar(
                    out=xpt[:, 1:H + 1, 1:W + 1],
                    in0=xs,
                    scalar1=1.0,
                    scalar2=None,
                    op0=mybir.AluOpType.mult,
                    op1=mybir.AluOpType.add,
                    accum_out=se[:, t, s:s + 1],
                )
                xp[s, t] = xpt

        # ---- gates -> block-diagonal stationary G (M is (s, q) sample-major)
        se_rep = singles.tile([128, 2, 64], FP32)
        for t in range(2):
            for s in range(BS):
                nc.vector.tensor_scalar(
                    out=se_rep[:, t, NQ * s:NQ * (s + 1)], in0=ones16,
                    scalar1=se[:, t, s:s + 1], scalar2=None,
                    op0=mybir.AluOpType.mult)
        L = psS.tile([128, 64], FP32, tag="ps", name="Lpsum")
        for t in range(2):
            nc.tensor.matmul(L, lhsT=rwx[:, t, :], rhs=se_rep[:, t, :],
                             start=(t == 0), stop=(t == 1))
        g0 = singles.tile([128, 64], FP32)
        nc.scalar.activation(out=g0, in_=L,
                             func=mybir.ActivationFunctionType.Sigmoid,
                             bias=rbx, scale=1.0)
        G = singles.tile([128, 64], BF16)
        nc.vector.tensor_tensor(out=G, in0=g0, in1=mask, op=mybir.AluOpType.mult)
        if dbg:
            nc.sync.dma_start(out=dbgG_d[:], in_=G)

        # ---- bank prefetch (slot-gated by the pool's 4 bufs)
        bkt = {}
        for w in range(6):
            for u in range(2):
                bk = bankp.tile([128, 8, NWIN], BF16, tag="bk",
                                name=f"bk{w}_{u}")
                nc.sync.dma_start(out=bk, in_=bank_d[:, w, 8 * u:8 * u + 8])
                bkt[w, u] = bk

        # ---- per half: synthesize w_dyn on PE, SBUF re-gather, then conv
        for half in range(2):
            stage = stagep.tile([128, 8, 3, NWIN], BF16, tag="stage",
                                name=f"stage{half}")
            for wloc in range(3):
                w = 3 * half + wloc
                bks = [bkt[w, 0], bkt[w, 1]]
                for clp in range(8):
                    # MMs for cl=clp (u=0) and cl=clp+8 (u=1) share one
                    # (128,384) PSUM tile via column-strip offsets; one
                    # drain covers both
                    ps = psS.tile([128, NWIN], FP32, tag="ps",
                                  name=f"ps{half}_{wloc}_{clp}")
                    nc.tensor.matmul(ps[0:64, :], lhsT=G,
                                     rhs=bks[0][:, clp, :],
                                     start=True, stop=True)
                    nc.tensor.matmul(ps[64:128, :], lhsT=G,
                                     rhs=bks[1][:, clp, :],
                                     start=True, stop=True)
                    dst = stage[:, clp, wloc, :]
                    if clp % 2 == 0:
                        nc.scalar.activation(
                            out=dst, in_=ps,
                            func=mybir.ActivationFunctionType.Copy)
                    else:
                        nc.vector.tensor_copy(out=dst, in_=ps)

            if dbg and half == 0:
                nc.sync.dma_start(out=dbgS_d[:], in_=stage)
            wd = {}
            for s in range(BS):
                for t in range(2):
                    wdt = wdynp.tile([128, 9, 128], BF16, tag=f"wd{half}{s}{t}")
                    # stage partition 64u+16s+8t+q' holds ci=(8t+q')*16+clp+8u
                    # for clp in the free dim; with the host-side ci
                    # permutation, partition d=64u+8q'+clp of the conv
                    # stationary IS that channel, so both dst slices are
                    # contiguous.
                    for u in range(2):
                        src = stage[64 * u + NQ * s + 8 * t:
                                    64 * u + NQ * s + 8 * t + 8]
                        eng = nc.sync if u == 0 else nc.scalar
                        eng.dma_start(out=wdt[64 * u:64 * (u + 1)], in_=src)
                    wd[s, t] = wdt
                    if dbg and half == 0 and s == 0 and t == 0:
                        nc.sync.dma_start(out=dbgW_d[:], in_=wdt)

            for s in range(BS):
                pst = [psC.tile([128, 14, W], FP32, tag="pc",
                                name=f"pc{half}_{s}_{c}") for c in range(2)]
                for t in range(2):
                    for k in range(9):
                        kh, kw = divmod(k, 3)
                        lw = wd[s, t][:, k, :]
                        for c in range(2):
                            rhs = xp[s, t][:, c * 14 + kh:c * 14 + kh + 14, kw:kw + W]
                            nc.tensor.matmul(
                                pst[c], lhsT=lw, rhs=rhs,
                                start=(t == 0 and k == 0),
                                stop=(t == 1 and k == 8),
                            )
                ot = outp.tile([128, 2, 14, W], FP32, tag="ot",
                               name=f"ot{half}_{s}")
                for c in range(2):
                    nc.scalar.activation(
                        out=ot[:, c], in_=pst[c],
                        func=mybir.ActivationFunctionType.Identity,
                        bias=biasT[:, half:half + 1], scale=1.0)
                nc.scalar.dma_start(
                    out=out_d[s, half * 128:(half + 1) * 128], in_=ot)
    nc.finalize()
    return nc


# partition d (within a 128-channel tile) holds channel perm[d]:
# d = 64u + 8q' + clp  <->  ci_lo = 16q' + 8u + clp
CI_PERM = np.array([(d % 64) // 8 * 16 + (d // 64) * 8 + d % 8
                    for d in range(128)])
CI_MAP = np.concatenate([CI_PERM, 128 + CI_PERM])


def _host_prep(route_w, route_b, weight, bias):
    """Host-side layout transforms (pure numpy, replicated to every core)."""
    We = np.ascontiguousarray(weight.T).reshape(NUM, C_OUT, C_IN, K, K)
    Wf = We.transpose(0, 2, 1, 3, 4)            # [e, ci, co, kh, kw]
    Wf = Wf.reshape(NUM, C_IN, 2, 128, 9)       # [e, ci, co_t, co_lo, khkw]
    Wf = Wf.transpose(0, 1, 2, 4, 3)            # [e, ci, co_t, khkw, co_lo]
    Wf = Wf.reshape(NUM, C_IN, F)               # f = co_t*1152 + khkw*128 + co_lo
    Bk = Wf.reshape(NUM, NQ, NQ, 6, NWIN)       # [e, q, cl, w, n]
    bank = np.ascontiguousarray(
        Bk.transpose(1, 0, 3, 2, 4).reshape(128, 6, NQ, NWIN)).astype(BF16_NP)

    rwx = np.ascontiguousarray(
        np.tile((route_w / (H * W)).T, (1, NQ))[CI_MAP]).astype(np.float32)
    rbx = np.tile(route_b, NQ).reshape(128, 1).astype(np.float32)
    # G column m = (s, q): q(m) = m % 16
    mask = (np.arange(128)[:, None] // 8 == np.arange(64)[None, :] % NQ
            ).astype(BF16_NP)
    bias2 = np.ascontiguousarray(bias.reshape(2, 128, 1)).astype(np.float32)
    return bank, rwx, rbx, mask, bias2


def _ensure_ntff_hook():
    """Provide antenv.axon_hooks (absent in this image) so trace=True works.

    The boot script ships a ctypes NTFF hook but can only register it through
    antenv.axon_hooks; shim that module and register the hook ourselves.
    """
    import sys
    import types
    try:
        from antenv.axon_hooks import get_axon_ntff_profile_hook  # noqa: F401
        return
    except ImportError:
        pass
    try:
        import antenv
        from trn_agent_boot.trn_boot import _ntff_profile_via_ctypes
    except ImportError:
        return
    mod = types.ModuleType("antenv.axon_hooks")
    holder = {"hook": None}
    mod.set_axon_ntff_profile_hook = lambda h: holder.__setitem__("hook", h)
    mod.get_axon_ntff_profile_hook = lambda: holder["hook"]
    sys.modules["antenv.axon_hooks"] = mod
    antenv.axon_hooks = mod
    mod.set_axon_ntff_profile_hook(
        _ntff_profile_via_ctypes("/opt/axon/libaxon_pjrt.so"))


_NC_CACHE = None


def kernel(inputs, route_w, route_b, weight, bias):
    global _NC_CACHE
    inputs = np.asarray(inputs, dtype=np.float32)
    route_w = np.asarray(route_w, dtype=np.float32)
    route_b = np.asarray(route_b, dtype=np.float32)
    weight = np.asarray(weight, dtype=np.float32)
    bias = np.asarray(bias, dtype=np.float32)

    bank, rwx, rbx, mask, bias2 = _host_prep(route_w, route_b, weight, bias)

    if _NC_CACHE is None:
        _NC_CACHE = build_nc()
    nc = _NC_CACHE

    shared = {"bank": bank, "rwx": rwx, "rbx": rbx, "mask": mask, "bias": bias2}
    x16 = inputs[:, CI_MAP].astype(BF16_NP)
    in_maps = [
        {"x": np.ascontiguousarray(x16[BS * c:BS * (c + 1)]), **shared}
        for c in range(N_CORES)
    ]
    trace = bool(int(os.environ.get("KERNEL_TRACE", "0")))
    if trace:
        _ensure_ntff_hook()
    res = run_bass_kernel_spmd(
        nc, in_maps, core_ids=list(range(N_CORES)), trace=trace,
        tmpdir=os.environ.get("KERNEL_TMPDIR"),
    )
    out = np.concatenate([res.results[c]["out"] for c in range(N_CORES)], axis=0)
    kernel.last_results = res
    return out


kernel.last_results = None


# revision 22
# speedup vs baseline: 1.2248x; 1.0588x over previous
"""Dynamic-conv (CondConv-style) kernel for Trainium2, 8 NeuronCores.

Problem: for each sample b:
    se     = global-avg-pool(x[b])                     (256,)
    gates  = sigmoid(se @ route_w.T + route_b)         (8,)
    w_dyn  = (gates @ weight.T).reshape(256,256,3,3)   per-sample 3x3 conv kernel
    out[b] = conv2d(x[b], w_dyn, pad=1) + bias         (256,28,28)

Sharding: data-parallel over batch, 4 samples per core; the expert weight
bank and routing weights are replicated.

Per-core plan:
  - x: DMA fp32 -> cast+zero-pad to bf16 (30x30) on DVE, fused free-dim
    sum (accum_out) to get the pooled `se`.
  - gates: tiny fp32 matmuls against a 16x-replicated routing matrix produce
    a 128x64 block of logits; sigmoid (ACT) + block-diagonal mask (DVE)
    yield a block-diagonal stationary G[(q,e),(s,q)] = gate[s,e] * (q==q').
  - kernel synthesis on the TensorEngine: the weight bank is laid out so a
    rhs tile's partition p=(q,e) carries 16 different ci-chunks of all 8
    experts; one matmul against G computes sum_e gate[s,e]*W_e for 16
    chunks x 4 samples at once (128-wide contraction, 128 elems/cycle).
  - PSUM (64,384) tiles drain (cast bf16) alternately on ACT/DVE into a
    (64, 16*3*384) SBUF stage; one SBUF->SBUF DMA per (sample, ci_tile)
    re-gathers w_dyn with ci on partitions -- the conv stationary layout.
    M is ordered sample-major so the re-gather source is a contiguous
    8-partition slice.
  - conv: 9 shifted matmuls (bf16) accumulating over (ci_tile, kh, kw) in
    PSUM; ACT adds bias on the PSUM->SBUF drain; DMA out fp32.
  - split into halves by output-channel block so conv(half A) overlaps the
    synthesis of half B; DMA issue is spread over sync/scalar/vector queues.
"""

import os
from contextlib import ExitStack

import ml_dtypes
import numpy as np

import concourse.bacc as bacc
import concourse.bass as bass
import concourse.mybir as mybir
import concourse.tile as tile
from concourse.bass_utils import run_bass_kernel_spmd

FP32 = mybir.dt.float32
BF16 = mybir.dt.bfloat16
BF16_NP = ml_dtypes.bfloat16

N_CORES = 8
B, C_IN, H, W = 32, 256, 28, 28
NUM, C_OUT, K = 8, 256, 3
BS = B // N_CORES          # samples per core = 4
NQ = 16                    # ci chunks in the synthesis contraction
F = 2304                   # f = co_t*1152 + khkw*128 + co_lo
NWIN = 384                 # synthesis matmul free size


def build_nc() -> bacc.Bacc:
    nc = bacc.Bacc("TRN2", target_bir_lowering=False, debug=False,
                   num_devices=N_CORES)

    x_d = nc.dram_tensor("x", [BS, C_IN, H, W], BF16, kind="ExternalInput")
    # bank[p=(q,e), w, cl, n] = W[e, ci=q*16+cl, f=w*384+n]; per-partition
    # contiguous so one DMA per w moves 12.3KB/partition runs.
    bank_d = nc.dram_tensor("bank", [128, 6, NQ, NWIN], BF16, kind="ExternalInput")
    rwx_d = nc.dram_tensor("rwx", [C_IN, 128], FP32, kind="ExternalInput")
    rbx_d = nc.dram_tensor("rbx", [128, 1], FP32, kind="ExternalInput")
    mask_d = nc.dram_tensor("mask", [128, 64], BF16, kind="ExternalInput")
    bias_d = nc.dram_tensor("bias", [2, 128, 1], FP32, kind="ExternalInput")
    out_d = nc.dram_tensor("out", [BS, C_OUT, H, W], FP32, kind="ExternalOutput")
    dbg = os.environ.get("KERNEL_DEBUG_TAPS")
    if dbg:
        dbgG_d = nc.dram_tensor("dbgG", [128, 64], BF16, kind="ExternalOutput")
        dbgS_d = nc.dram_tensor("dbgS", [128, 8, 3, 384], BF16, kind="ExternalOutput")
        dbgW_d = nc.dram_tensor("dbgW", [128, 9, 128], BF16, kind="ExternalOutput")

    with tile.TileContext(nc) as tc, ExitStack() as ctx:
        singles = ctx.enter_context(tc.tile_pool(name="singles", bufs=1))
        xstage = ctx.enter_context(tc.tile_pool(name="xstage", bufs=8))
        xpadp = ctx.enter_context(tc.tile_pool(name="xpadp", bufs=1))
        bankp = ctx.enter_context(tc.tile_pool(name="bankp", bufs=12))
        stagep = ctx.enter_context(tc.tile_pool(name="stagep", bufs=2))
        wdynp = ctx.enter_context(tc.tile_pool(name="wdynp", bufs=1))
        outp = ctx.enter_context(tc.tile_pool(name="outp", bufs=2))
        psS = ctx.enter_context(tc.tile_pool(name="psS", bufs=4, space="PSUM"))
        psC = ctx.enter_context(tc.tile_pool(name="psC", bufs=4, space="PSUM"))

        # ---- replicated constants
        rwx = singles.tile([128, 2, 128], FP32)
        for t in range(2):
            nc.sync.dma_start(out=rwx[:, t, :], in_=rwx_d[t * 128:(t + 1) * 128, :])
        rbx = singles.tile([128, 1], FP32)
        nc.sync.dma_start(out=rbx, in_=rbx_d[:])
        mask = singles.tile([128, 64], BF16)
        nc.sync.dma_start(out=mask, in_=mask_d[:])
        biasT = singles.tile([128, 2], FP32)
        for t in range(2):
            nc.sync.dma_start(out=biasT[:, t:t + 1], in_=bias_d[t])
        ones16 = singles.tile([128, NQ], FP32)
        nc.vector.memset(ones16, 1.0)
        warm = singles.tile([128, 1], FP32)
        nc.scalar.activation(out=warm, in_=ones16[:, 0:1],
                             func=mybir.ActivationFunctionType.Sigmoid)
        nc.scalar.activation(out=warm, in_=warm,
                             func=mybir.ActivationFunctionType.Identity,
                             bias=warm, scale=1.0)
        nc.scalar.activation(out=warm, in_=warm,
                             func=mybir.ActivationFunctionType.Copy)

        # ---- x in; cast+pad to bf16 with fused pooling
        se = singles.tile([128, 2, BS], FP32)
        xp = {}
        for s in range(BS):
            for t in range(2):
                xs = xstage.tile([128, H, W], BF16, tag="xs")
                nc.sync.dma_start(out=xs, in_=x_d[s, t * 128:(t + 1) * 128])
                xpt = xpadp.tile([128, H + 2, W + 2], BF16, tag=f"xp{s}{t}")
                nc.gpsimd.memset(xpt, 0.0)
                nc.vector.tensor_scalar(
                    out=xpt[:, 1:H + 1, 1:W + 1],
                    in0=xs,
                    scalar1=1.0,
                    scalar2=None,
                    op0=mybir.AluOpType.mult,
                    op1=mybir.AluOpType.add,
                    accum_out=se[:, t, s:s + 1],
                )
                xp[s, t] = xpt

        # ---- gates -> block-diagonal stationary G (M is (s, q) sample-major)
        se_rep = singles.tile([128, 2, 64], FP32)
        for t in range(2):
            for s in range(BS):
                nc.vector.tensor_scalar(
                    out=se_rep[:, t, NQ * s:NQ * (s + 1)], in0=ones16,
                    scalar1=se[:, t, s:s + 1], scalar2=None,
                    op0=mybir.AluOpType.mult)
        L = psS.tile([128, 64], FP32, tag="ps", name="Lpsum")
        for t in range(2):
            nc.tensor.matmul(L, lhsT=rwx[:, t, :], rhs=se_rep[:, t, :],
                             start=(t == 0), stop=(t == 1))
        g0 = singles.tile([128, 64], FP32)
        nc.scalar.activation(out=g0, in_=L,
                             func=mybir.ActivationFunctionType.Sigmoid,
                             bias=rbx, scale=1.0)
        G = singles.tile([128, 64], BF16)
        nc.vector.tensor_tensor(out=G, in0=g0, in1=mask, op=mybir.AluOpType.mult)
        if dbg:
            nc.sync.dma_start(out=dbgG_d[:], in_=G)

        # ---- bank prefetch (slot-gated by the pool's 4 bufs)
        bkt = {}
        for w in range(6):
            for u in range(2):
                bk = bankp.tile([128, 8, NWIN], BF16, tag="bk",
                                name=f"bk{w}_{u}")
                nc.sync.dma_start(out=bk, in_=bank_d[:, w, 8 * u:8 * u + 8])
                bkt[w, u] = bk

        # ---- per half: synthesize w_dyn on PE, SBUF re-gather, then conv
        for half in range(2):
            stage = stagep.tile([128, 8, 3, NWIN], BF16, tag="stage",
                                name=f"stage{half}")
            for wloc in range(3):
                w = 3 * half + wloc
                bks = [bkt[w, 0], bkt[w, 1]]
                for clp in range(8):
                    # MMs for cl=clp (u=0) and cl=clp+8 (u=1) share one
                    # (128,384) PSUM tile via column-strip offsets; one
                    # drain covers both
                    ps = psS.tile([128, NWIN], FP32, tag="ps",
                                  name=f"ps{half}_{wloc}_{clp}")
                    nc.tensor.matmul(ps[0:64, :], lhsT=G,
                                     rhs=bks[0][:, clp, :],
                                     start=True, stop=True)
                    nc.tensor.matmul(ps[64:128, :], lhsT=G,
                                     rhs=bks[1][:, clp, :],
                                     start=True, stop=True)
                    dst = stage[:, clp, wloc, :]
                    if clp % 2 == 0:
                        nc.scalar.activation(
                            out=dst, in_=ps,
                            func=mybir.ActivationFunctionType.Copy)
                    else:
                        nc.vector.tensor_copy(out=dst, in_=ps)

            if dbg and half == 0:
                nc.sync.dma_start(out=dbgS_d[:], in_=stage)
            wd = {}
            for s in range(BS):
                for t in range(2):
                    wdt = wdynp.tile([128, 9, 128], BF16, tag=f"wd{half}{s}{t}")
                    # stage partition 64u+16s+8t+q' holds ci=(8t+q')*16+clp+8u
                    # for clp in the free dim; with the host-side ci
                    # permutation, partition d=64u+8q'+clp of the conv
                    # stationary IS that channel, so both dst slices are
                    # contiguous.
                    for u in range(2):
                        src = stage[64 * u + NQ * s + 8 * t:
                                    64 * u + NQ * s + 8 * t + 8]
                        eng = nc.sync if u == 0 else nc.scalar
                        eng.dma_start(out=wdt[64 * u:64 * (u + 1)], in_=src)
                    wd[s, t] = wdt
                    if dbg and half == 0 and s == 0 and t == 0:
                        nc.sync.dma_start(out=dbgW_d[:], in_=wdt)

            for s in range(BS):
                pst = [psC.tile([128, 14, W], FP32, tag="pc",
                                name=f"pc{half}_{s}_{c}") for c in range(2)]
                for t in range(2):
                    for k in range(9):
                        kh, kw = divmod(k, 3)
                        lw = wd[s, t][:, k, :]
                        for c in range(2):
                            rhs = xp[s, t][:, c * 14 + kh:c * 14 + kh + 14, kw:kw + W]
                            nc.tensor.matmul(
                                pst[c], lhsT=lw, rhs=rhs,
                                start=(t == 0 and k == 0),
                                stop=(t == 1 and k == 8),
                            )
                ot = outp.tile([128, 2, 14, W], FP32, tag="ot",
                               name=f"ot{half}_{s}")
                for c in range(2):
                    nc.scalar.activation(
                        out=ot[:, c], in_=pst[c],
                        func=mybir.ActivationFunctionType.Identity,
                        bias=biasT[:, half:half + 1], scale=1.0)
                nc.scalar.dma_start(
                    out=out_d[s, half * 128:(half + 1) * 128], in_=ot)
    nc.finalize()
    return nc


# partition d (within a 128-channel tile) holds channel perm[d]:
# d = 64u + 8q' + clp  <->  ci_lo = 16q' + 8u + clp
CI_PERM = np.array([(d % 64) // 8 * 16 + (d // 64) * 8 + d % 8
                    for d in range(128)])
CI_MAP = np.concatenate([CI_PERM, 128 + CI_PERM])


def _host_prep(route_w, route_b, weight, bias):
    """Host-side layout transforms (pure numpy, replicated to every core)."""
    We = np.ascontiguousarray(weight.T).reshape(NUM, C_OUT, C_IN, K, K)
    Wf = We.transpose(0, 2, 1, 3, 4)            # [e, ci, co, kh, kw]
    Wf = Wf.reshape(NUM, C_IN, 2, 128, 9)       # [e, ci, co_t, co_lo, khkw]
    Wf = Wf.transpose(0, 1, 2, 4, 3)            # [e, ci, co_t, khkw, co_lo]
    Wf = Wf.reshape(NUM, C_IN, F)               # f = co_t*1152 + khkw*128 + co_lo
    Bk = Wf.reshape(NUM, NQ, NQ, 6, NWIN)       # [e, q, cl, w, n]
    bank = np.ascontiguousarray(
        Bk.transpose(1, 0, 3, 2, 4).reshape(128, 6, NQ, NWIN)).astype(BF16_NP)

    rwx = np.ascontiguousarray(
        np.tile((route_w / (H * W)).T, (1, NQ))[CI_MAP]).astype(np.float32)
    rbx = np.tile(route_b, NQ).reshape(128, 1).astype(np.float32)
    # G column m = (s, q): q(m) = m % 16
    mask = (np.arange(128)[:, None] // 8 == np.arange(64)[None, :] % NQ
            ).astype(BF16_NP)
    bias2 = np.ascontiguousarray(bias.reshape(2, 128, 1)).astype(np.float32)
    return bank, rwx, rbx, mask, bias2


def _ensure_ntff_hook():
    """Provide antenv.axon_hooks (absent in this image) so trace=True works.

    The boot script ships a ctypes NTFF hook but can only register it through
    antenv.axon_hooks; shim that module and register the hook ourselves.
    """
    import sys
    import types
    try:
        from antenv.axon_hooks import get_axon_ntff_profile_hook  # noqa: F401
        return
    except ImportError:
        pass
    try:
        import antenv
        from trn_agent_boot.trn_boot import _ntff_profile_via_ctypes
    except ImportError:
        return
    mod = types.ModuleType("antenv.axon_hooks")
    holder = {"hook": None}
    mod.set_axon_ntff_profile_hook = lambda h: holder.__setitem__("hook", h)
    mod.get_axon_ntff_profile_hook = lambda: holder["hook"]
    sys.modules["antenv.axon_hooks"] = mod
    antenv.axon_hooks = mod
    mod.set_axon_ntff_profile_hook(
        _ntff_profile_via_ctypes("/opt/axon/libaxon_pjrt.so"))


_NC_CACHE = None


def kernel(inputs, route_w, route_b, weight, bias):
    global _NC_CACHE
    inputs = np.asarray(inputs, dtype=np.float32)
    route_w = np.asarray(route_w, dtype=np.float32)
    route_b = np.asarray(route_b, dtype=np.float32)
    weight = np.asarray(weight, dtype=np.float32)
    bias = np.asarray(bias, dtype=np.float32)

    bank, rwx, rbx, mask, bias2 = _host_prep(route_w, route_b, weight, bias)

    if _NC_CACHE is None:
        _NC_CACHE = build_nc()
    nc = _NC_CACHE

    shared = {"bank": bank, "rwx": rwx, "rbx": rbx, "mask": mask, "bias": bias2}
    x16 = inputs[:, CI_MAP].astype(BF16_NP)
    in_maps = [
        {"x": np.ascontiguousarray(x16[BS * c:BS * (c + 1)]), **shared}
        for c in range(N_CORES)
    ]
    trace = bool(int(os.environ.get("KERNEL_TRACE", "0")))
    if trace:
        _ensure_ntff_hook()
    res = run_bass_kernel_spmd(
        nc, in_maps, core_ids=list(range(N_CORES)), trace=trace,
        tmpdir=os.environ.get("KERNEL_TMPDIR"),
    )
    out = np.concatenate([res.results[c]["out"] for c in range(N_CORES)], axis=0)
    kernel.last_results = res
    return out


kernel.last_results = None
